# revision 32
# baseline (speedup 1.0000x reference)
"""DeltaNet decode step on 8 Trainium2 NeuronCores (tensor-parallel over heads).

Contract: kernel(**inputs) takes the FULL unsharded inputs (numpy arrays,
same keys as the reference setup_inputs()) and returns the FULL output
[1, 4096, 1, 1] float32.

Sharding (8 cores, 16 heads -> 2 heads/core):
  - Wq/Wk rows, q/k conv weights+caches: 512 rows per core
  - Wv rows, v conv weights+caches, Wo columns: 1024 per core
  - state: 2 heads per core
  - output: each core computes a partial [4096] projection; host all-reduces.

Device kernel (memory-bound streaming, ~16.4MB/core):
  - Wq/Wk/Wv stream as single fp8-e4m3 (x128 scale), consumed by DoubleRow
    matmuls: each [128, 2, 512] rhs carries TWO 128-row contraction chunks;
    lhsT carries the matching h chunks as fp8 (hi, lo) pairs in the M dim
    (hi = e4m3(16h), lo = e4m3(64*(16h - hi))), folded by scaled K=2
    matmuls into columns. End-to-end rel err ~1.7e-2 (gate 2e-2).
  - Wo streams in bf16 with ov cast to bf16.
  - l2-normalization of q/k heads is deferred: the state matvecs run on
    raw silu(conv()) vectors as 4 batched bf16 [2,512] row matmuls
    (lhsT = (k,q) column pairs), and the 1/||.|| factors fold into the
    per-head combine scalars (a*rk, a*rq, b*dot*rq*rk).
  - ACT table loads are hoisted off the critical path (Silu preloaded via
    a dummy op before the post-stream silu(v)).
"""

import sys
import types

sys.path.insert(0, "/opt/trn_rl_repo")

import numpy as np
import ml_dtypes

import concourse.bass as bass
import concourse.mybir as mybir
import concourse.tile as tile
from concourse import bacc
from concourse.bass_utils import run_bass_kernel_spmd

BF16 = ml_dtypes.bfloat16
E4 = ml_dtypes.float8_e4m3
F32 = mybir.dt.float32
BF = mybir.dt.bfloat16
F8 = mybir.dt.float8e4
AF = mybir.ActivationFunctionType
OP = mybir.AluOpType
PM = mybir.MatmulPerfMode

H = 4096
QK = 4096
VD = 8192
EPS = 1e-6
NCORES = 8
HPC = 2          # heads per core
RQ = 512         # q/k rows per core
RV = 1024        # v rows / Wo cols per core

SW = 128.0       # fp8 weight scale
SH_HI = 16.0     # fp8 h hi scale
SH_LO = 64.0     # fp8 h lo extra scale
# fold scales: x = row_hi/(SW*SH_HI) + row_lo/(SW*SH_HI*SH_LO); both are
# powers of two -> exact in bf16
FS_HI = 1.0 / (SW * SH_HI)
FS_LO = 1.0 / (SW * SH_HI * SH_LO)

_CACHE = {}


def _ensure_ntff_hook():
    """Install the axon NTFF profile hook shim (antenv.axon_hooks is absent
    in this image). Harmless if profiling is never requested."""
    if "antenv.axon_hooks" in sys.modules:
        return
    try:
        import antenv
        mod = types.ModuleType("antenv.axon_hooks")
        mod._hook = None
        mod.set_axon_ntff_profile_hook = lambda h: setattr(mod, "_hook", h)
        mod.get_axon_ntff_profile_hook = lambda: mod._hook
        sys.modules["antenv.axon_hooks"] = mod
        antenv.axon_hooks = mod
        from trn_agent_boot.trn_boot import _ntff_profile_via_ctypes
        mod._hook = _ntff_profile_via_ctypes("/opt/axon/libaxon_pjrt.so")
    except Exception:
        pass


def _build_nc():
    nc = bacc.Bacc(None)

    d = {}
    d["wqk8"] = nc.dram_tensor("wqk8", [4, 128, 8192], F8, kind="ExternalInput")
    d["wv8"] = nc.dram_tensor("wv8", [4, 128, 8192], F8, kind="ExternalInput")
    d["wo16"] = nc.dram_tensor("wo16", [4, 128, 8192], BF, kind="ExternalInput")
    d["hf8"] = nc.dram_tensor("hf8", [128, 64], F8, kind="ExternalInput")
    d["wab"] = nc.dram_tensor("wab", [128, 128], F32, kind="ExternalInput")
    d["hrep"] = nc.dram_tensor("hrep", [128, 128], F32, kind="ExternalInput")
    d["state16"] = nc.dram_tensor("state16", [128, 2048], BF, kind="ExternalInput")
    d["qkcache"] = nc.dram_tensor("qkcache", [128, 24], F32, kind="ExternalInput")
    d["qkconvw"] = nc.dram_tensor("qkconvw", [128, 32], F32, kind="ExternalInput")
    d["vcache"] = nc.dram_tensor("vcache", [128, 24], F32, kind="ExternalInput")
    d["vconvw"] = nc.dram_tensor("vconvw", [128, 32], F32, kind="ExternalInput")
    d["fsc32"] = nc.dram_tensor("fsc32", [2, 1], F32, kind="ExternalInput")
    d["eye16"] = nc.dram_tensor("eye16", [2, 2], BF, kind="ExternalInput")
    out_d = nc.dram_tensor("out", [2, H], F32, kind="ExternalOutput")

    with tile.TileContext(nc) as tc:
        with (
            tc.tile_pool(name="smalls", bufs=1) as sm,
            tc.tile_pool(name="wp8", bufs=6) as wp8,
            tc.tile_pool(name="wpo", bufs=4) as wpo,
            tc.tile_pool(name="psum", bufs=8, space="PSUM") as pm,
        ):
            def emit():
                # ---- small input DMAs (SWDGE keeps the HWDGE ring clear) ----
                hf8 = sm.tile([128, 2, 32], F8, tag="hf8")
                nc.gpsimd.dma_start(
                    out=hf8[:], in_=d["hf8"][:].rearrange("p (i m) -> p i m", i=2))
                wab = sm.tile([128, 128], F32, tag="wab")
                hrep = sm.tile([128, 128], F32, tag="hrep")
                st16 = sm.tile([128, 2048], BF, tag="st16")
                qkca = sm.tile([128, 24], F32, tag="qkca")
                qkcw = sm.tile([128, 32], F32, tag="qkcw")
                vca = sm.tile([128, 24], F32, tag="vca")
                vcw = sm.tile([128, 32], F32, tag="vcw")
                fsc = sm.tile([2, 1], F32, tag="fsc")
                eye = sm.tile([2, 2], BF, tag="eye")
                for t, src in [(wab, "wab"), (hrep, "hrep"), (fsc, "fsc32"),
                               (eye, "eye16"), (st16, "state16"),
                               (qkca, "qkcache"), (qkcw, "qkconvw"),
                               (vca, "vcache"), (vcw, "vconvw")]:
                    nc.gpsimd.dma_start(out=t[:], in_=d[src][:])
                ones = sm.tile([1, 128], F32, tag="ones")
                nc.vector.memset(ones[:], 1.0)
                onesc = sm.tile([128, 1], F32, tag="onesc")
                nc.vector.memset(onesc[:], 1.0)
                epst = sm.tile([1, 1], F32, tag="epst")
                nc.vector.memset(epst[:], EPS)

                # ---- psum tiles (allocation order fixes pool-slot reuse) ----
                ps_ab = pm.tile([1, 4], F32, tag="ps", name="ps_ab")
                ps_q = pm.tile([2, 512], F32, tag="ps", name="ps_q")
                ps_k = pm.tile([2, 512], F32, tag="ps", name="ps_k")
                ps_v0 = pm.tile([2, 512], F32, tag="ps", name="ps_v0")
                ps_v1 = pm.tile([2, 512], F32, tag="ps", name="ps_v1")
                t_col = pm.tile([128, 8], F32, tag="ps", name="t_col")
                t_row = pm.tile([1, 8], F32, tag="ps", name="t_row")
                ps_st0 = pm.tile([2, 512], F32, tag="ps", name="ps_st0")
                ps_st1 = pm.tile([2, 512], F32, tag="ps", name="ps_st1")
                ps_stc = pm.tile([128, 16], F32, tag="ps", name="ps_stc")

                # ---- sbuf chain tiles ----
                ab = sm.tile([1, 4], F32, tag="ab")
                qsb = sm.tile([2, 512], F32, tag="qsb")
                ksb = sm.tile([2, 512], F32, tag="ksb")
                qkcol = sm.tile([128, 8], F32, tag="qkcol")
                qacc = sm.tile([128, 8], F32, tag="qacc")
                qtmp = sm.tile([128, 8], F32, tag="qtmp")
                x1 = sm.tile([128, 8], F32, tag="x1")
                x116 = sm.tile([128, 8], BF, tag="x116")
                sq = sm.tile([128, 8], F32, tag="sq")
                ssr = sm.tile([1, 8], F32, tag="ssr")
                ssh = sm.tile([1, 4], F32, tag="ssh")
                rin = sm.tile([1, 4], F32, tag="rin")
                dm = sm.tile([128, 4], F32, tag="dm")
                dotr = sm.tile([1, 4], F32, tag="dotr")
                dot = sm.tile([1, 2], F32, tag="dot")
                aq2 = sm.tile([1, 2], F32, tag="aq2")
                ak2 = sm.tile([1, 2], F32, tag="ak2")
                bd = sm.tile([1, 2], F32, tag="bd")
                bdak = sm.tile([1, 2], F32, tag="bdak")
                abc6 = sm.tile([128, 6], F32, tag="abc6")
                vacc = sm.tile([128, 8], F32, tag="vacc")
                vtmp = sm.tile([128, 8], F32, tag="vtmp")
                stsb0 = sm.tile([2, 512], BF, tag="stsb0")
                stsb1 = sm.tile([2, 512], BF, tag="stsb1")
                vsb = sm.tile([2, 1024], F32, tag="vsb")
                vcol = sm.tile([128, 8], F32, tag="vcol")
                v1c = sm.tile([128, 8], F32, tag="v1c")
                u1c = sm.tile([128, 8], F32, tag="u1c")
                u2c = sm.tile([128, 8], F32, tag="u2c")
                errc = sm.tile([128, 4], F32, tag="errc")
                t1c = sm.tile([128, 4], F32, tag="t1c")
                ov16 = sm.tile([128, 16], BF, tag="ov16")
                dum = sm.tile([1, 1], F32, tag="dum")
                out_sb = sm.tile([2, H], F32, tag="out_sb")

                # ---- injected work: alpha/beta + conv cache taps ----
                def pre_ab():
                    # hrep[p, 4cc+j] = h[cc*128+p]; wab[p, 4cc+j] = Wab[j, ...]
                    abm = sm.tile([128, 128], F32, tag="abm")
                    nc.vector.tensor_mul(abm[:], wab[:], hrep[:])
                    abr = sm.tile([128, 4], F32, tag="abr")
                    nc.vector.reduce_sum(
                        abr[:],
                        abm[:].rearrange("p (cc f) -> p f cc", f=4),
                        axis=mybir.AxisListType.X)
                    nc.tensor.matmul(ps_ab[0:1, :], onesc[:, 0:1], abr[:],
                                     start=True, stop=True)
                    nc.scalar.activation(ab[:], ps_ab[:], AF.Sigmoid)

                def pre_taps():
                    # q/k conv cache taps -> qacc; v conv cache taps -> vacc
                    nc.vector.tensor_mul(qacc[:], qkca[:, 0:8], qkcw[:, 0:8])
                    for tpi in (1, 2):
                        nc.vector.tensor_mul(qtmp[:], qkca[:, 8 * tpi:8 * tpi + 8],
                                             qkcw[:, 8 * tpi:8 * tpi + 8])
                        nc.vector.tensor_add(qacc[:], qacc[:], qtmp[:])
                    nc.vector.tensor_mul(vacc[:], vca[:, 0:8], vcw[:, 0:8])
                    for tpi in (1, 2):
                        nc.vector.tensor_mul(vtmp[:], vca[:, 8 * tpi:8 * tpi + 8],
                                             vcw[:, 8 * tpi:8 * tpi + 8])
                        nc.vector.tensor_add(vacc[:], vacc[:], vtmp[:])

                # ---- fp8 DoubleRow streaming matvec ----
                def stream8(dram, ps0, ps1, inject=None):
                    """dram [4, 128, 8192] fp8, layout (d, p, (pr two rh r)).
                    rh=0 -> ps0[2,512], rh=1 -> ps1[2,512], M=2 (h hi, lo)."""
                    last = None
                    for dd in range(4):
                        t = wp8.tile([128, 8192], F8, tag="w8", name="w8t")
                        nc.sync.dma_start(out=t[:], in_=dram[dd])
                        tv = t[:].rearrange(
                            "p (pr two rh r) -> p pr two rh r",
                            pr=4, two=2, r=512)
                        last = tv
                        for pr in range(4):
                            pair = 4 * dd + pr
                            lh = hf8[:, 0:2, 2 * pair:2 * pair + 2]
                            nc.tensor.matmul(
                                ps0[0:2, :], lh, tv[:, pr, 0:2, 0, :],
                                start=(pair == 0), stop=(pair == 15),
                                perf_mode=PM.DoubleRow)
                            nc.tensor.matmul(
                                ps1[0:2, :], lh, tv[:, pr, 0:2, 1, :],
                                start=(pair == 0), stop=(pair == 15),
                                perf_mode=PM.DoubleRow)
                        if inject and dd in inject:
                            inject[dd]()
                    return last

                def chain_pe_0():
                    # scaled hi/lo fold + row->column (K=2, f32)
                    for c in range(4):
                        nc.tensor.matmul(t_col[:, c:c + 1],
                                         ksb[0:2, 128 * c:128 * c + 128],
                                         fsc[0:2, 0:1], start=True, stop=True)
                        nc.tensor.matmul(t_col[:, 4 + c:5 + c],
                                         qsb[0:2, 128 * c:128 * c + 128],
                                         fsc[0:2, 0:1], start=True, stop=True)
                    nc.vector.tensor_copy(qkcol[:], t_col[:])
                    # conv tap3 + silu in columns (raw, un-normalized)
                    nc.vector.tensor_mul(qtmp[:], qkcol[:], qkcw[:, 24:32])
                    nc.vector.tensor_add(qtmp[:], qacc[:], qtmp[:])
                    nc.scalar.activation(x1[:], qtmp[:], AF.Silu)
                    nc.vector.tensor_copy(x116[:], x1[:])
                    nc.vector.tensor_mul(sq[:], x1[:], x1[:])

                def chain_pe_1():
                    # per-column sum of squares -> per-head 1/||.||
                    nc.tensor.matmul(t_row[0:1, :], onesc[:, 0:1], sq[:],
                                     start=True, stop=True)
                    nc.vector.tensor_copy(ssr[:], t_row[0:1, :])
                    nc.vector.reduce_sum(
                        ssh[0:1, 0:4],
                        ssr[0:1, :].rearrange("a (g t) -> a g t", t=2),
                        axis=mybir.AxisListType.X)
                    srt = sm.tile([1, 4], F32, tag="srt")
                    nc.scalar.activation(srt[:], ssh[:], AF.Sqrt,
                                         bias=epst[0:1, 0:1])
                    nc.vector.reciprocal(rin[:], srt[:])
                    # raw q.k dot per head
                    nc.vector.tensor_mul(dm[:], x1[:, 4:8], x1[:, 0:4])

                def chain_pe_2():
                    nc.tensor.matmul(t_row[0:1, 0:4], onesc[:, 0:1], dm[:],
                                     start=True, stop=True)
                    # state matvecs, batched: lhsT = (k,q) column pairs of raw
                    # x1 (bf16), rhs = state rows -> ps_st[hh] rows (ks, qs)
                    xv = x116[:].rearrange("p (g c) -> p c g", c=4)
                    for hh in range(HPC):
                        pst = ps_st0 if hh == 0 else ps_st1
                        for d2 in range(2):
                            blk = 2 * hh + d2
                            nc.tensor.matmul(
                                pst[0:2, :], xv[:, 2 * hh + d2, 0:2],
                                st16[:, 512 * blk:512 * blk + 512],
                                start=(d2 == 0), stop=(d2 == 1))
                    nc.vector.tensor_copy(dotr[:], t_row[0:1, 0:4])
                    nc.vector.reduce_sum(
                        dot[0:1, 0:2],
                        dotr[0:1, :].rearrange("a (g t) -> a g t", t=2),
                        axis=mybir.AxisListType.X)
                    # per-head scalars: aq = a*rq, bd = b*dot_raw*rk*rq,
                    # bdak = bd*a*rk
                    nc.vector.tensor_mul(aq2[:], ab[0:1, 0:2], rin[0:1, 2:4])
                    nc.vector.tensor_mul(ak2[:], ab[0:1, 0:2], rin[0:1, 0:2])
                    nc.vector.tensor_mul(dot[:], dot[:], rin[0:1, 0:2])
                    nc.vector.tensor_mul(dot[:], dot[:], rin[0:1, 2:4])
                    nc.vector.tensor_mul(bd[:], ab[0:1, 2:4], dot[:])
                    nc.vector.tensor_mul(bdak[:], bd[:], ak2[:])
                    # silu table preload for the post-stream silu(v)
                    nc.scalar.activation(dum[:], epst[:], AF.Silu)
                    # broadcast [aq0 aq1 bdak0 bdak1 bd0 bd1] to 128 partitions
                    for j in range(2):
                        nc.tensor.matmul(t_col[:, j:j + 1], ones[0:1, :],
                                         aq2[0:1, j:j + 1], start=True, stop=True)
                        nc.tensor.matmul(t_col[:, 2 + j:3 + j], ones[0:1, :],
                                         bdak[0:1, j:j + 1], start=True, stop=True)
                        nc.tensor.matmul(t_col[:, 4 + j:5 + j], ones[0:1, :],
                                         bd[0:1, j:j + 1], start=True, stop=True)
                    nc.vector.tensor_copy(abc6[:], t_col[:, 0:6])
                    # fold state rows to columns and build the early half of
                    # the o-projection lhsT: u1 = aq*qs - bd*ak*ks
                    nc.vector.tensor_copy(stsb0[:], ps_st0[0:2, :])
                    nc.vector.tensor_copy(stsb1[:], ps_st1[0:2, :])
                    for hh in range(HPC):
                        ssb = stsb0 if hh == 0 else stsb1
                        for c in range(4):
                            nc.tensor.matmul(
                                ps_stc[:, 8 * hh + 2 * c:8 * hh + 2 * c + 2],
                                ssb[0:2, 128 * c:128 * c + 128],
                                eye[0:2, 0:2], start=True, stop=True)
                    stc = ps_stc[:].rearrange("p (hh c n) -> p hh n c", hh=2, n=2)
                    for hh in range(HPC):
                        nc.vector.tensor_scalar(out=t1c[:], in0=stc[:, hh, 1, :],
                                                scalar1=abc6[:, hh:hh + 1],
                                                scalar2=None, op0=OP.mult)
                        nc.vector.tensor_scalar(out=errc[:], in0=stc[:, hh, 0, :],
                                                scalar1=abc6[:, 2 + hh:3 + hh],
                                                scalar2=None, op0=OP.mult)
                        nc.vector.tensor_sub(u1c[:, 4 * hh:4 * hh + 4],
                                             t1c[:], errc[:])
                    # u1 -> even lhsT columns of the o-projection
                    nc.vector.tensor_copy(
                        ov16[:].rearrange("p (j m) -> p m j", m=2)[:, 0, :],
                        u1c[:])

                # ---- phase 0: PE warm-up during the DMA ramp (memset-fed
                # dummy matmuls; the real accumulations reset psum with
                # start=True so these are harmless) ----
                wsc = sm.tile([128, 512], BF, tag="wsc")
                nc.vector.memset(wsc[:], 0.5)
                lsc = sm.tile([128, 2], BF, tag="lsc")
                nc.vector.memset(lsc[:], 0.5)
                for i in range(18):
                    nc.tensor.matmul((ps_q if i % 2 else ps_k)[0:2, :],
                                     lsc[:, 0:2], wsc[:],
                                     start=True, stop=True)

                # ---- phase 1: q/k matvec (rh=0 -> q rows, rh=1 -> k rows) ----
                stream8(d["wqk8"], ps_q, ps_k,
                        inject={0: pre_ab, 1: pre_taps})
                nc.vector.tensor_copy(qsb[:], ps_q[0:2, :])
                nc.vector.tensor_copy(ksb[:], ps_k[0:2, :])

                # ---- phase 2: v matvec with injected chain ----
                vlast = stream8(d["wv8"], ps_v0, ps_v1,
                                inject={0: chain_pe_0, 1: chain_pe_1,
                                        2: chain_pe_2})

                # ---- phase 3: post-stream chain (v only) ----
                nc.vector.tensor_copy(vsb[0:2, 0:512], ps_v0[0:2, :])
                nc.vector.tensor_copy(vsb[0:2, 512:1024], ps_v1[0:2, :])
                # v hi/lo fold to columns (f32)
                for j in range(8):
                    nc.tensor.matmul(t_col[:, j:j + 1],
                                     vsb[0:2, 128 * j:128 * j + 128],
                                     fsc[0:2, 0:1], start=True, stop=True)
                nc.vector.tensor_copy(vcol[:], t_col[:])
                # v conv tap3 + silu, then u2 = bd*v1 -> odd lhsT columns
                nc.vector.tensor_mul(vtmp[:], vcol[:], vcw[:, 24:32])
                nc.vector.tensor_add(vtmp[:], vacc[:], vtmp[:])
                nc.scalar.activation(v1c[:], vtmp[:], AF.Silu)
                for hh in range(HPC):
                    nc.vector.tensor_scalar(out=u2c[:, 4 * hh:4 * hh + 4],
                                            in0=v1c[:, 4 * hh:4 * hh + 4],
                                            scalar1=abc6[:, 4 + hh:5 + hh],
                                            scalar2=None, op0=OP.mult)
                nc.vector.tensor_copy(
                    ov16[:].rearrange("p (j m) -> p m j", m=2)[:, 1, :],
                    u2c[:])

                # ---- phase 4: output projection (bf16, M=2: u1, u2) ----
                ps_o = [pm.tile([2, 512], F32, tag="ps", name=f"ps_o{i}")
                        for i in range(8)]
                for dd in range(4):
                    t = wpo.tile([128, 8192], BF, tag="wo", name="wot")
                    nc.sync.dma_start(out=t[:], in_=d["wo16"][dd])
                    for i in range(2):
                        j = 2 * dd + i
                        for it in range(8):
                            nc.tensor.matmul(
                                ps_o[it][0:2, :], ov16[:, 2 * j:2 * j + 2],
                                t[:, 4096 * i + 512 * it:4096 * i + 512 * it + 512],
                                start=(j == 0), stop=(j == 7))
                for it in range(8):
                    dst = out_sb[0:2, 512 * it:512 * it + 512]
                    if it % 2 == 0:
                        nc.vector.tensor_copy(dst, ps_o[it][0:2, :])
                    else:
                        nc.scalar.copy(dst, ps_o[it][0:2, :])
                nc.sync.dma_start(out=out_d[:], in_=out_sb[:])

            emit()

    nc.finalize()
    return nc


def _prep_in_maps(inputs):
    f32 = np.float32
    hid = np.asarray(inputs["hidden_states"], f32)[0, :, 0, 0]     # [4096]
    Wq = np.asarray(inputs["Wq"], f32)
    Wk = np.asarray(inputs["Wk"], f32)
    Wv = np.asarray(inputs["Wv"], f32)
    Wo = np.asarray(inputs["Wo"], f32)
    Wa = np.asarray(inputs["Wa"], f32)
    Wb = np.asarray(inputs["Wb"], f32)
    qcw = np.asarray(inputs["q_conv_w"], f32)[0]                   # [QK, 4]
    kcw = np.asarray(inputs["k_conv_w"], f32)[0]
    vcw = np.asarray(inputs["v_conv_w"], f32)[0]                   # [VD, 4]
    qca = np.asarray(inputs["q_cache"], f32)[0]                    # [QK, 3]
    kca = np.asarray(inputs["k_cache"], f32)[0]
    vca = np.asarray(inputs["v_cache"], f32)[0]                    # [VD, 3]
    state = np.asarray(inputs["state"], f32)[0]                    # [16,256,512]

    # h fp8 hi/lo pair, [128, 64]: col (two*32 + pair*2 + m),
    # value row j = (pair*2 + two)*128 + p
    h_hi8 = (hid * SH_HI).astype(E4)
    h_lo8 = ((hid * SH_HI - h_hi8.astype(f32)) * SH_LO).astype(E4)
    hp = np.stack([h_hi8, h_lo8], -1).reshape(16, 2, 128, 2)  # pair,two,p,m
    hf8 = np.ascontiguousarray(
        hp.transpose(2, 1, 0, 3).reshape(128, 64))            # p,(two pair m)

    # h replicated x4 for the DVE alpha/beta matvec: hrep[p, 4cc+j] = h[cc*128+p]
    hrep = np.ascontiguousarray(
        np.repeat(hid.reshape(32, 128).T[:, :, None], 4, axis=2).reshape(128, 128))

    def pack8(wt):
        """wt [4096, 1024] fp8 (contraction-major) -> [4, 128, 8192] with
        tile layout (d, p, (pr two rh r))."""
        a = wt.reshape(4, 4, 2, 128, 2, 512)      # d pr two p rh r
        return np.ascontiguousarray(
            a.transpose(0, 3, 1, 2, 4, 5).reshape(4, 128, 8192))

    in_maps = []
    for c in range(NCORES):
        rq = slice(c * RQ, (c + 1) * RQ)
        rv = slice(c * RV, (c + 1) * RV)
        wqk = np.concatenate([Wq[rq], Wk[rq]], axis=0)             # [1024, 4096]
        wqk8 = pack8(np.ascontiguousarray((wqk.T * SW)).astype(E4))
        wv8 = pack8(np.ascontiguousarray((Wv[rv].T * SW)).astype(E4))
        # Wo columns rv, transposed [1024, 4096] bf16, tiles (d, p, (i r))
        wot = np.ascontiguousarray(Wo[:, rv].T).astype(BF16)
        wo16 = np.ascontiguousarray(
            wot.reshape(4, 2, 128, 4096).transpose(0, 2, 1, 3).reshape(4, 128, 8192))

        wab = np.concatenate([Wa[2 * c:2 * c + 2], Wb[2 * c:2 * c + 2]], 0)
        wab_sb = np.ascontiguousarray(
            wab.reshape(4, 32, 128).transpose(2, 1, 0).reshape(128, 128))
        st_sb = np.ascontiguousarray(
            state[2 * c:2 * c + 2].reshape(2, 2, 128, 512)
            .transpose(2, 0, 1, 3).reshape(128, 2048)).astype(BF16)

        # q/k conv in column layout [128, 8*taps]: per tap, cols 0-3 = k
        # chunks (k idx 128c+p), cols 4-7 = q chunks
        qk_ca = np.concatenate(
            [np.concatenate([kca[rq, t].reshape(4, 128).T,
                             qca[rq, t].reshape(4, 128).T], 1)
             for t in range(3)], 1)
        qk_cw = np.concatenate(
            [np.concatenate([kcw[rq, t].reshape(4, 128).T,
                             qcw[rq, t].reshape(4, 128).T], 1)
             for t in range(4)], 1)
        # v conv in column layout [128, 8*taps]: vcol[p, 8t+cc] = v[128cc+p, t]
        v_ca = np.ascontiguousarray(
            vca[rv].reshape(8, 128, 3).transpose(1, 2, 0).reshape(128, 24))
        v_cw = np.ascontiguousarray(
            vcw[rv].reshape(8, 128, 4).transpose(1, 2, 0).reshape(128, 32))

        in_maps.append({
            "wqk8": wqk8, "wv8": wv8, "wo16": wo16,
            "hf8": hf8, "wab": wab_sb, "hrep": hrep, "state16": st_sb,
            "qkcache": np.ascontiguousarray(qk_ca),
            "qkconvw": np.ascontiguousarray(qk_cw),
            "vcache": v_ca, "vconvw": v_cw,
            "fsc32": np.array([[FS_HI], [FS_LO]], f32),
            "eye16": np.eye(2, dtype=f32).astype(BF16),
        })
    return in_maps


def _run(inputs, trace=False, tmpdir=None):
    _ensure_ntff_hook()
    if "nc" not in _CACHE:
        _CACHE["nc"] = _build_nc()
    nc = _CACHE["nc"]
    in_maps = _prep_in_maps(inputs)
    res = run_bass_kernel_spmd(nc, in_maps, list(range(NCORES)),
                               trace=trace, tmpdir=tmpdir)
    acc = np.zeros(H, np.float64)
    for c in range(NCORES):
        r = res.results[c]["out"].astype(np.float64)
        acc += r[0] + r[1]
    out = acc.astype(np.float32).reshape(1, H, 1, 1)
    return out, res


def kernel(**inputs):
    out, _ = _run(inputs, trace=False)
    return out


def kernel_traced(tmpdir=None, **inputs):
    return _run(inputs, trace=True, tmpdir=tmpdir)


# revision 37
# speedup vs baseline: 1.0181x; 1.0181x over previous
"""DeltaNet decode step on 8 Trainium2 NeuronCores (tensor-parallel over heads).

Contract: kernel(**inputs) takes the FULL unsharded inputs (numpy arrays,
same keys as the reference setup_inputs()) and returns the FULL output
[1, 4096, 1, 1] float32.

Sharding (8 cores, 16 heads -> 2 heads/core):
  - Wq/Wk rows, q/k conv weights+caches: 512 rows per core
  - Wv rows, v conv weights+caches, Wo columns: 1024 per core
  - state: 2 heads per core
  - output: each core computes a partial [4096] projection; host all-reduces.

Device kernel (memory-bound streaming, ~16.4MB/core):
  - Wq/Wk/Wv stream as single fp8-e4m3 (x128 scale), consumed by DoubleRow
    matmuls: each [128, 2, 512] rhs carries TWO 128-row contraction chunks;
    lhsT carries the matching h chunks as fp8 (hi, lo) pairs in the M dim
    (hi = e4m3(16h), lo = e4m3(64*(16h - hi))), folded by scaled K=2
    matmuls into columns. End-to-end rel err ~1.7e-2 (gate 2e-2).
  - Wo streams in bf16 with ov cast to bf16.
  - l2-normalization of q/k heads is deferred: the state matvecs run on
    raw silu(conv()) vectors as 4 batched bf16 [2,512] row matmuls
    (lhsT = (k,q) column pairs), and the 1/||.|| factors fold into the
    per-head combine scalars (a*rk, a*rq, b*dot*rq*rk).
  - ACT table loads are hoisted off the critical path (Silu preloaded via
    a dummy op before the post-stream silu(v)).
"""

import sys
import types

sys.path.insert(0, "/opt/trn_rl_repo")

import numpy as np
import ml_dtypes

import concourse.bass as bass
import concourse.mybir as mybir
import concourse.tile as tile
from concourse import bacc
from concourse.bass_utils import run_bass_kernel_spmd

BF16 = ml_dtypes.bfloat16
E4 = ml_dtypes.float8_e4m3
F32 = mybir.dt.float32
BF = mybir.dt.bfloat16
F8 = mybir.dt.float8e4
AF = mybir.ActivationFunctionType
OP = mybir.AluOpType
PM = mybir.MatmulPerfMode

H = 4096
QK = 4096
VD = 8192
EPS = 1e-6
NCORES = 8
HPC = 2          # heads per core
RQ = 512         # q/k rows per core
RV = 1024        # v rows / Wo cols per core

SW = 128.0       # fp8 weight scale
SH_HI = 16.0     # fp8 h hi scale
SH_LO = 64.0     # fp8 h lo extra scale
# fold scales: x = row_hi/(SW*SH_HI) + row_lo/(SW*SH_HI*SH_LO); both are
# powers of two -> exact in bf16
FS_HI = 1.0 / (SW * SH_HI)
FS_LO = 1.0 / (SW * SH_HI * SH_LO)

_CACHE = {}


def _ensure_ntff_hook():
    """Install the axon NTFF profile hook shim (antenv.axon_hooks is absent
    in this image). Harmless if profiling is never requested."""
    if "antenv.axon_hooks" in sys.modules:
        return
    try:
        import antenv
        mod = types.ModuleType("antenv.axon_hooks")
        mod._hook = None
        mod.set_axon_ntff_profile_hook = lambda h: setattr(mod, "_hook", h)
        mod.get_axon_ntff_profile_hook = lambda: mod._hook
        sys.modules["antenv.axon_hooks"] = mod
        antenv.axon_hooks = mod
        from trn_agent_boot.trn_boot import _ntff_profile_via_ctypes
        mod._hook = _ntff_profile_via_ctypes("/opt/axon/libaxon_pjrt.so")
    except Exception:
        pass


def _build_nc():
    nc = bacc.Bacc(None)

    d = {}
    d["wqk8"] = nc.dram_tensor("wqk8", [4, 128, 8192], F8, kind="ExternalInput")
    d["wv8"] = nc.dram_tensor("wv8", [4, 128, 8192], F8, kind="ExternalInput")
    d["wo16"] = nc.dram_tensor("wo16", [4, 128, 8192], BF, kind="ExternalInput")
    d["hf8"] = nc.dram_tensor("hf8", [128, 64], F8, kind="ExternalInput")
    d["wab"] = nc.dram_tensor("wab", [128, 128], F32, kind="ExternalInput")
    d["hrep"] = nc.dram_tensor("hrep", [128, 128], F32, kind="ExternalInput")
    d["state16"] = nc.dram_tensor("state16", [128, 2048], BF, kind="ExternalInput")
    d["qkcache"] = nc.dram_tensor("qkcache", [128, 24], F32, kind="ExternalInput")
    d["qkconvw"] = nc.dram_tensor("qkconvw", [128, 32], F32, kind="ExternalInput")
    d["vcache"] = nc.dram_tensor("vcache", [128, 24], F32, kind="ExternalInput")
    d["vconvw"] = nc.dram_tensor("vconvw", [128, 32], F32, kind="ExternalInput")
    d["fsc32"] = nc.dram_tensor("fsc32", [2, 1], F32, kind="ExternalInput")
    d["eye16"] = nc.dram_tensor("eye16", [2, 2], BF, kind="ExternalInput")
    out_d = nc.dram_tensor("out", [2, H], F32, kind="ExternalOutput")

    with tile.TileContext(nc) as tc:
        with (
            tc.tile_pool(name="smalls", bufs=1) as sm,
            tc.tile_pool(name="wp8", bufs=6) as wp8,
            tc.tile_pool(name="wpo", bufs=4) as wpo,
            tc.tile_pool(name="psum", bufs=8, space="PSUM") as pm,
        ):
            def emit():
                # ---- small input DMAs (SWDGE keeps the HWDGE ring clear) ----
                hf8 = sm.tile([128, 2, 32], F8, tag="hf8")
                nc.gpsimd.dma_start(
                    out=hf8[:], in_=d["hf8"][:].rearrange("p (i m) -> p i m", i=2))
                wab = sm.tile([128, 128], F32, tag="wab")
                hrep = sm.tile([128, 128], F32, tag="hrep")
                st16 = sm.tile([128, 2048], BF, tag="st16")
                qkca = sm.tile([128, 24], F32, tag="qkca")
                qkcw = sm.tile([128, 32], F32, tag="qkcw")
                vca = sm.tile([128, 24], F32, tag="vca")
                vcw = sm.tile([128, 32], F32, tag="vcw")
                fsc = sm.tile([2, 1], F32, tag="fsc")
                eye = sm.tile([2, 2], BF, tag="eye")
                for t, src in [(wab, "wab"), (hrep, "hrep"), (fsc, "fsc32"),
                               (eye, "eye16"), (st16, "state16"),
                               (qkca, "qkcache"), (qkcw, "qkconvw"),
                               (vca, "vcache"), (vcw, "vconvw")]:
                    nc.gpsimd.dma_start(out=t[:], in_=d[src][:])
                ones = sm.tile([1, 128], F32, tag="ones")
                nc.vector.memset(ones[:], 1.0)
                onesc = sm.tile([128, 1], F32, tag="onesc")
                nc.vector.memset(onesc[:], 1.0)
                epst = sm.tile([1, 1], F32, tag="epst")
                nc.vector.memset(epst[:], EPS)

                # ---- psum tiles (allocation order fixes pool-slot reuse) ----
                ps_ab = pm.tile([1, 4], F32, tag="ps", name="ps_ab")
                ps_q = pm.tile([2, 512], F32, tag="ps", name="ps_q")
                ps_k = pm.tile([2, 512], F32, tag="ps", name="ps_k")
                ps_v0 = pm.tile([2, 512], F32, tag="ps", name="ps_v0")
                ps_v1 = pm.tile([2, 512], F32, tag="ps", name="ps_v1")
                t_col = pm.tile([128, 8], F32, tag="ps", name="t_col")
                t_row = pm.tile([1, 8], F32, tag="ps", name="t_row")
                ps_st0 = pm.tile([2, 512], F32, tag="ps", name="ps_st0")
                ps_st1 = pm.tile([2, 512], F32, tag="ps", name="ps_st1")
                ps_stc = pm.tile([128, 16], F32, tag="ps", name="ps_stc")

                # ---- sbuf chain tiles ----
                ab = sm.tile([1, 4], F32, tag="ab")
                qsb = sm.tile([2, 512], F32, tag="qsb")
                ksb = sm.tile([2, 512], F32, tag="ksb")
                qkcol = sm.tile([128, 8], F32, tag="qkcol")
                qacc = sm.tile([128, 8], F32, tag="qacc")
                qtmp = sm.tile([128, 8], F32, tag="qtmp")
                x1 = sm.tile([128, 8], F32, tag="x1")
                x116 = sm.tile([128, 8], BF, tag="x116")
                sq = sm.tile([128, 8], F32, tag="sq")
                ssr = sm.tile([1, 8], F32, tag="ssr")
                ssh = sm.tile([1, 4], F32, tag="ssh")
                rin = sm.tile([1, 4], F32, tag="rin")
                dm = sm.tile([128, 4], F32, tag="dm")
                dotr = sm.tile([1, 4], F32, tag="dotr")
                dot = sm.tile([1, 2], F32, tag="dot")
                aq2 = sm.tile([1, 2], F32, tag="aq2")
                ak2 = sm.tile([1, 2], F32, tag="ak2")
                bd = sm.tile([1, 2], F32, tag="bd")
                bdak = sm.tile([1, 2], F32, tag="bdak")
                abc6 = sm.tile([128, 6], F32, tag="abc6")
                vacc = sm.tile([128, 8], F32, tag="vacc")
                vtmp = sm.tile([128, 8], F32, tag="vtmp")
                stsb0 = sm.tile([2, 512], BF, tag="stsb0")
                stsb1 = sm.tile([2, 512], BF, tag="stsb1")
                vsb = sm.tile([2, 1024], F32, tag="vsb")
                vcol = sm.tile([128, 8], F32, tag="vcol")
                v1c = sm.tile([128, 8], F32, tag="v1c")
                u1c = sm.tile([128, 8], F32, tag="u1c")
                u2c = sm.tile([128, 8], F32, tag="u2c")
                errc = sm.tile([128, 4], F32, tag="errc")
                t1c = sm.tile([128, 4], F32, tag="t1c")
                ov16 = sm.tile([128, 16], BF, tag="ov16")
                dum = sm.tile([1, 1], F32, tag="dum")
                out_sb = sm.tile([2, H], F32, tag="out_sb")

                # ---- injected work: alpha/beta + conv cache taps ----
                def pre_ab():
                    # hrep[p, 4cc+j] = h[cc*128+p]; wab[p, 4cc+j] = Wab[j, ...]
                    abm = sm.tile([128, 128], F32, tag="abm")
                    nc.vector.tensor_mul(abm[:], wab[:], hrep[:])
                    abr = sm.tile([128, 4], F32, tag="abr")
                    nc.vector.reduce_sum(
                        abr[:],
                        abm[:].rearrange("p (cc f) -> p f cc", f=4),
                        axis=mybir.AxisListType.X)
                    nc.tensor.matmul(ps_ab[0:1, :], onesc[:, 0:1], abr[:],
                                     start=True, stop=True)
                    nc.scalar.activation(ab[:], ps_ab[:], AF.Sigmoid)

                def pre_taps():
                    # q/k conv cache taps -> qacc; v conv cache taps -> vacc
                    nc.vector.tensor_mul(qacc[:], qkca[:, 0:8], qkcw[:, 0:8])
                    for tpi in (1, 2):
                        nc.vector.tensor_mul(qtmp[:], qkca[:, 8 * tpi:8 * tpi + 8],
                                             qkcw[:, 8 * tpi:8 * tpi + 8])
                        nc.vector.tensor_add(qacc[:], qacc[:], qtmp[:])
                    nc.vector.tensor_mul(vacc[:], vca[:, 0:8], vcw[:, 0:8])
                    for tpi in (1, 2):
                        nc.vector.tensor_mul(vtmp[:], vca[:, 8 * tpi:8 * tpi + 8],
                                             vcw[:, 8 * tpi:8 * tpi + 8])
                        nc.vector.tensor_add(vacc[:], vacc[:], vtmp[:])

                # ---- fp8 DoubleRow streaming matvec ----
                def stream8(dram, ps0, ps1, inject=None):
                    """dram [4, 128, 8192] fp8, layout (d, p, (pr two rh r)).
                    rh=0 -> ps0[2,512], rh=1 -> ps1[2,512], M=2 (h hi, lo)."""
                    last = None
                    for dd in range(4):
                        t = wp8.tile([128, 8192], F8, tag="w8", name="w8t")
                        nc.sync.dma_start(out=t[:], in_=dram[dd])
                        tv = t[:].rearrange(
                            "p (pr two rh r) -> p pr two rh r",
                            pr=4, two=2, r=512)
                        last = tv
                        for pr in range(4):
                            pair = 4 * dd + pr
                            lh = hf8[:, 0:2, 2 * pair:2 * pair + 2]
                            nc.tensor.matmul(
                                ps0[0:2, :], lh, tv[:, pr, 0:2, 0, :],
                                start=(pair == 0), stop=(pair == 15),
                                perf_mode=PM.DoubleRow)
                            nc.tensor.matmul(
                                ps1[0:2, :], lh, tv[:, pr, 0:2, 1, :],
                                start=(pair == 0), stop=(pair == 15),
                                perf_mode=PM.DoubleRow)
                        if inject and dd in inject:
                            inject[dd]()
                    return last

                def chain_pe_0():
                    # scaled hi/lo fold + row->column (K=2, f32)
                    for c in range(4):
                        nc.tensor.matmul(t_col[:, c:c + 1],
                                         ksb[0:2, 128 * c:128 * c + 128],
                                         fsc[0:2, 0:1], start=True, stop=True)
                        nc.tensor.matmul(t_col[:, 4 + c:5 + c],
                                         qsb[0:2, 128 * c:128 * c + 128],
                                         fsc[0:2, 0:1], start=True, stop=True)
                    # conv tap3 + silu in columns (raw, un-normalized)
                    nc.vector.tensor_mul(qtmp[:], t_col[:], qkcw[:, 24:32])
                    nc.vector.tensor_add(qtmp[:], qacc[:], qtmp[:])
                    nc.scalar.activation(x1[:], qtmp[:], AF.Silu)
                    nc.vector.tensor_copy(x116[:], x1[:])
                    nc.vector.tensor_mul(sq[:], x1[:], x1[:])

                def chain_pe_1():
                    # per-column sum of squares -> per-head 1/||.||
                    nc.tensor.matmul(t_row[0:1, :], onesc[:, 0:1], sq[:],
                                     start=True, stop=True)
                    nc.vector.reduce_sum(
                        ssh[0:1, 0:4],
                        t_row[0:1, :].rearrange("a (g t) -> a g t", t=2),
                        axis=mybir.AxisListType.X)
                    srt = sm.tile([1, 4], F32, tag="srt")
                    nc.scalar.activation(srt[:], ssh[:], AF.Sqrt,
                                         bias=epst[0:1, 0:1])
                    nc.vector.reciprocal(rin[:], srt[:])
                    # raw q.k dot per head
                    nc.vector.tensor_mul(dm[:], x1[:, 4:8], x1[:, 0:4])

                def chain_pe_2():
                    nc.tensor.matmul(t_row[0:1, 0:4], onesc[:, 0:1], dm[:],
                                     start=True, stop=True)
                    # state matvecs, batched: lhsT = (k,q) column pairs of raw
                    # x1 (bf16), rhs = state rows -> ps_st[hh] rows (ks, qs)
                    xv = x116[:].rearrange("p (g c) -> p c g", c=4)
                    for hh in range(HPC):
                        pst = ps_st0 if hh == 0 else ps_st1
                        for d2 in range(2):
                            blk = 2 * hh + d2
                            nc.tensor.matmul(
                                pst[0:2, :], xv[:, 2 * hh + d2, 0:2],
                                st16[:, 512 * blk:512 * blk + 512],
                                start=(d2 == 0), stop=(d2 == 1))
                    nc.vector.reduce_sum(
                        dot[0:1, 0:2],
                        t_row[0:1, 0:4].rearrange("a (g t) -> a g t", t=2),
                        axis=mybir.AxisListType.X)
                    # per-head scalars: aq = a*rq, bd = b*dot_raw*rk*rq,
                    # bdak = bd*a*rk
                    nc.vector.tensor_mul(aq2[:], ab[0:1, 0:2], rin[0:1, 2:4])
                    nc.vector.tensor_mul(ak2[:], ab[0:1, 0:2], rin[0:1, 0:2])
                    nc.vector.tensor_mul(dot[:], dot[:], rin[0:1, 0:2])
                    nc.vector.tensor_mul(dot[:], dot[:], rin[0:1, 2:4])
                    nc.vector.tensor_mul(bd[:], ab[0:1, 2:4], dot[:])
                    nc.vector.tensor_mul(bdak[:], bd[:], ak2[:])
                    # silu table preload for the post-stream silu(v)
                    nc.scalar.activation(dum[:], epst[:], AF.Silu)
                    # broadcast [aq0 aq1 bdak0 bdak1 bd0 bd1] to 128 partitions
                    for j in range(2):
                        nc.tensor.matmul(t_col[:, j:j + 1], ones[0:1, :],
                                         aq2[0:1, j:j + 1], start=True, stop=True)
                        nc.tensor.matmul(t_col[:, 2 + j:3 + j], ones[0:1, :],
                                         bdak[0:1, j:j + 1], start=True, stop=True)
                        nc.tensor.matmul(t_col[:, 4 + j:5 + j], ones[0:1, :],
                                         bd[0:1, j:j + 1], start=True, stop=True)
                    nc.vector.tensor_copy(abc6[:], t_col[:, 0:6])
                    # fold state rows to columns and build the early half of
                    # the o-projection lhsT: u1 = aq*qs - bd*ak*ks
                    nc.vector.tensor_copy(stsb0[:], ps_st0[0:2, :])
                    nc.vector.tensor_copy(stsb1[:], ps_st1[0:2, :])
                    for hh in range(HPC):
                        ssb = stsb0 if hh == 0 else stsb1
                        for c in range(4):
                            nc.tensor.matmul(
                                ps_stc[:, 8 * hh + 2 * c:8 * hh + 2 * c + 2],
                                ssb[0:2, 128 * c:128 * c + 128],
                                eye[0:2, 0:2], start=True, stop=True)
                    stc = ps_stc[:].rearrange("p (hh c n) -> p hh n c", hh=2, n=2)
                    for hh in range(HPC):
                        nc.vector.tensor_scalar(out=t1c[:], in0=stc[:, hh, 1, :],
                                                scalar1=abc6[:, hh:hh + 1],
                                                scalar2=None, op0=OP.mult)
                        nc.vector.tensor_scalar(out=errc[:], in0=stc[:, hh, 0, :],
                                                scalar1=abc6[:, 2 + hh:3 + hh],
                                                scalar2=None, op0=OP.mult)
                        nc.vector.tensor_sub(u1c[:, 4 * hh:4 * hh + 4],
                                             t1c[:], errc[:])
                    # u1 -> even lhsT columns of the o-projection
                    nc.vector.tensor_copy(
                        ov16[:].rearrange("p (j m) -> p m j", m=2)[:, 0, :],
                        u1c[:])

                # ---- phase 1: q/k matvec (rh=0 -> q rows, rh=1 -> k rows) ----
                stream8(d["wqk8"], ps_q, ps_k,
                        inject={0: pre_ab, 1: pre_taps})
                nc.vector.tensor_copy(qsb[:], ps_q[0:2, :])
                nc.vector.tensor_copy(ksb[:], ps_k[0:2, :])

                # ---- phase 2: v matvec with injected chain ----
                vlast = stream8(d["wv8"], ps_v0, ps_v1,
                                inject={0: chain_pe_0, 1: chain_pe_1,
                                        2: chain_pe_2})

                # ---- phase 3: post-stream chain (v only), pipelined per head
                # so the o-projection can start on head0's tiles sooner ----
                nc.vector.tensor_copy(vsb[0:2, 0:512], ps_v0[0:2, :])
                nc.scalar.copy(vsb[0:2, 512:1024], ps_v1[0:2, :])
                ovv = ov16[:].rearrange("p (j m) -> p m j", m=2)
                for hh in range(HPC):
                    for c in range(4):
                        j = 4 * hh + c
                        nc.tensor.matmul(t_col[:, j:j + 1],
                                         vsb[0:2, 128 * j:128 * j + 128],
                                         fsc[0:2, 0:1], start=True, stop=True)
                    sl = slice(4 * hh, 4 * hh + 4)
                    nc.vector.tensor_mul(vtmp[:, sl], t_col[:, sl],
                                         vcw[:, 24 + 4 * hh:28 + 4 * hh])
                    nc.vector.tensor_add(vtmp[:, sl], vacc[:, sl], vtmp[:, sl])
                    nc.scalar.activation(v1c[:, sl], vtmp[:, sl], AF.Silu)
                    nc.vector.tensor_scalar(out=u2c[:, sl], in0=v1c[:, sl],
                                            scalar1=abc6[:, 4 + hh:5 + hh],
                                            scalar2=None, op0=OP.mult)
                    nc.vector.tensor_copy(ovv[:, 1, sl], u2c[:, sl])

                # ---- phase 4: output projection (bf16, M=2: u1, u2) ----
                ps_o = [pm.tile([2, 512], F32, tag="ps", name=f"ps_o{i}")
                        for i in range(8)]
                for dd in range(4):
                    t = wpo.tile([128, 8192], BF, tag="wo", name="wot")
                    nc.sync.dma_start(out=t[:], in_=d["wo16"][dd])
                    for i in range(2):
                        j = 2 * dd + i
                        for it in range(8):
                            nc.tensor.matmul(
                                ps_o[it][0:2, :], ov16[:, 2 * j:2 * j + 2],
                                t[:, 4096 * i + 512 * it:4096 * i + 512 * it + 512],
                                start=(j == 0), stop=(j == 7))
                for it in range(8):
                    dst = out_sb[0:2, 512 * it:512 * it + 512]
                    if it % 2 == 0:
                        nc.vector.tensor_copy(dst, ps_o[it][0:2, :])
                    else:
                        nc.scalar.copy(dst, ps_o[it][0:2, :])
                nc.sync.dma_start(out=out_d[:], in_=out_sb[:])

            emit()

    nc.finalize()
    return nc


def _prep_in_maps(inputs):
    f32 = np.float32
    hid = np.asarray(inputs["hidden_states"], f32)[0, :, 0, 0]     # [4096]
    Wq = np.asarray(inputs["Wq"], f32)
    Wk = np.asarray(inputs["Wk"], f32)
    Wv = np.asarray(inputs["Wv"], f32)
    Wo = np.asarray(inputs["Wo"], f32)
    Wa = np.asarray(inputs["Wa"], f32)
    Wb = np.asarray(inputs["Wb"], f32)
    qcw = np.asarray(inputs["q_conv_w"], f32)[0]                   # [QK, 4]
    kcw = np.asarray(inputs["k_conv_w"], f32)[0]
    vcw = np.asarray(inputs["v_conv_w"], f32)[0]                   # [VD, 4]
    qca = np.asarray(inputs["q_cache"], f32)[0]                    # [QK, 3]
    kca = np.asarray(inputs["k_cache"], f32)[0]
    vca = np.asarray(inputs["v_cache"], f32)[0]                    # [VD, 3]
    state = np.asarray(inputs["state"], f32)[0]                    # [16,256,512]

    # h fp8 hi/lo pair, [128, 64]: col (two*32 + pair*2 + m),
    # value row j = (pair*2 + two)*128 + p
    h_hi8 = (hid * SH_HI).astype(E4)
    h_lo8 = ((hid * SH_HI - h_hi8.astype(f32)) * SH_LO).astype(E4)
    hp = np.stack([h_hi8, h_lo8], -1).reshape(16, 2, 128, 2)  # pair,two,p,m
    hf8 = np.ascontiguousarray(
        hp.transpose(2, 1, 0, 3).reshape(128, 64))            # p,(two pair m)

    # h replicated x4 for the DVE alpha/beta matvec: hrep[p, 4cc+j] = h[cc*128+p]
    hrep = np.ascontiguousarray(
        np.repeat(hid.reshape(32, 128).T[:, :, None], 4, axis=2).reshape(128, 128))

    def pack8(wt):
        """wt [4096, 1024] fp8 (contraction-major) -> [4, 128, 8192] with
        tile layout (d, p, (pr two rh r))."""
        a = wt.reshape(4, 4, 2, 128, 2, 512)      # d pr two p rh r
        return np.ascontiguousarray(
            a.transpose(0, 3, 1, 2, 4, 5).reshape(4, 128, 8192))

    in_maps = []
    for c in range(NCORES):
        rq = slice(c * RQ, (c + 1) * RQ)
        rv = slice(c * RV, (c + 1) * RV)
        wqk = np.concatenate([Wq[rq], Wk[rq]], axis=0)             # [1024, 4096]
        wqk8 = pack8(np.ascontiguousarray((wqk.T * SW)).astype(E4))
        wv8 = pack8(np.ascontiguousarray((Wv[rv].T * SW)).astype(E4))
        # Wo columns rv, transposed [1024, 4096] bf16, tiles (d, p, (i r))
        wot = np.ascontiguousarray(Wo[:, rv].T).astype(BF16)
        wo16 = np.ascontiguousarray(
            wot.reshape(4, 2, 128, 4096).transpose(0, 2, 1, 3).reshape(4, 128, 8192))

        wab = np.concatenate([Wa[2 * c:2 * c + 2], Wb[2 * c:2 * c + 2]], 0)
        wab_sb = np.ascontiguousarray(
            wab.reshape(4, 32, 128).transpose(2, 1, 0).reshape(128, 128))
        st_sb = np.ascontiguousarray(
            state[2 * c:2 * c + 2].reshape(2, 2, 128, 512)
            .transpose(2, 0, 1, 3).reshape(128, 2048)).astype(BF16)

        # q/k conv in column layout [128, 8*taps]: per tap, cols 0-3 = k
        # chunks (k idx 128c+p), cols 4-7 = q chunks
        qk_ca = np.concatenate(
            [np.concatenate([kca[rq, t].reshape(4, 128).T,
                             qca[rq, t].reshape(4, 128).T], 1)
             for t in range(3)], 1)
        qk_cw = np.concatenate(
            [np.concatenate([kcw[rq, t].reshape(4, 128).T,
                             qcw[rq, t].reshape(4, 128).T], 1)
             for t in range(4)], 1)
        # v conv in column layout [128, 8*taps]: vcol[p, 8t+cc] = v[128cc+p, t]
        v_ca = np.ascontiguousarray(
            vca[rv].reshape(8, 128, 3).transpose(1, 2, 0).reshape(128, 24))
        v_cw = np.ascontiguousarray(
            vcw[rv].reshape(8, 128, 4).transpose(1, 2, 0).reshape(128, 32))

        in_maps.append({
            "wqk8": wqk8, "wv8": wv8, "wo16": wo16,
            "hf8": hf8, "wab": wab_sb, "hrep": hrep, "state16": st_sb,
            "qkcache": np.ascontiguousarray(qk_ca),
            "qkconvw": np.ascontiguousarray(qk_cw),
            "vcache": v_ca, "vconvw": v_cw,
            "fsc32": np.array([[FS_HI], [FS_LO]], f32),
            "eye16": np.eye(2, dtype=f32).astype(BF16),
        })
    return in_maps


def _run(inputs, trace=False, tmpdir=None):
    _ensure_ntff_hook()
    if "nc" not in _CACHE:
        _CACHE["nc"] = _build_nc()
    nc = _CACHE["nc"]
    in_maps = _prep_in_maps(inputs)
    res = run_bass_kernel_spmd(nc, in_maps, list(range(NCORES)),
                               trace=trace, tmpdir=tmpdir)
    acc = np.zeros(H, np.float64)
    for c in range(NCORES):
        r = res.results[c]["out"].astype(np.float64)
        acc += r[0] + r[1]
    out = acc.astype(np.float32).reshape(1, H, 1, 1)
    return out, res


def kernel(**inputs):
    out, _ = _run(inputs, trace=False)
    return out


def kernel_traced(tmpdir=None, **inputs):
    return _run(inputs, trace=True, tmpdir=tmpdir)


# revision 38
# speedup vs baseline: 1.1033x; 1.0837x over previous
"""DeltaNet decode step on 8 Trainium2 NeuronCores (tensor-parallel over heads).

Contract: kernel(**inputs) takes the FULL unsharded inputs (numpy arrays,
same keys as the reference setup_inputs()) and returns the FULL output
[1, 4096, 1, 1] float32.

Sharding (8 cores, 16 heads -> 2 heads/core):
  - Wq/Wk rows, q/k conv weights+caches: 512 rows per core
  - Wv rows, v conv weights+caches, Wo columns: 1024 per core
  - state: 2 heads per core
  - output: each core computes a partial [4096] projection; host all-reduces.

Device kernel (memory-bound streaming, ~16.4MB/core):
  - Wq/Wk/Wv stream as single fp8-e4m3 (x128 scale), consumed by DoubleRow
    matmuls: each [128, 2, 512] rhs carries TWO 128-row contraction chunks;
    lhsT carries the matching h chunks as fp8 (hi, lo) pairs in the M dim
    (hi = e4m3(16h), lo = e4m3(64*(16h - hi))), folded by scaled K=2
    matmuls into columns. End-to-end rel err ~1.7e-2 (gate 2e-2).
  - Wo streams in bf16 with ov cast to bf16.
  - l2-normalization of q/k heads is deferred: the state matvecs run on
    raw silu(conv()) vectors as 4 batched bf16 [2,512] row matmuls
    (lhsT = (k,q) column pairs), and the 1/||.|| factors fold into the
    per-head combine scalars (a*rk, a*rq, b*dot*rq*rk).
  - ACT table loads are hoisted off the critical path (Silu preloaded via
    a dummy op before the post-stream silu(v)).
"""

import sys
import types

sys.path.insert(0, "/opt/trn_rl_repo")

import numpy as np
import ml_dtypes

import concourse.bass as bass
import concourse.mybir as mybir
import concourse.tile as tile
from concourse import bacc
from concourse.bass_utils import run_bass_kernel_spmd

BF16 = ml_dtypes.bfloat16
E4 = ml_dtypes.float8_e4m3
F32 = mybir.dt.float32
BF = mybir.dt.bfloat16
F8 = mybir.dt.float8e4
AF = mybir.ActivationFunctionType
OP = mybir.AluOpType
PM = mybir.MatmulPerfMode

H = 4096
QK = 4096
VD = 8192
EPS = 1e-6
NCORES = 8
HPC = 2          # heads per core
RQ = 512         # q/k rows per core
RV = 1024        # v rows / Wo cols per core

SW = 128.0       # fp8 weight scale
SH_HI = 16.0     # fp8 h hi scale
SH_LO = 64.0     # fp8 h lo extra scale
# fold scales: x = row_hi/(SW*SH_HI) + row_lo/(SW*SH_HI*SH_LO); both are
# powers of two -> exact in bf16
FS_HI = 1.0 / (SW * SH_HI)
FS_LO = 1.0 / (SW * SH_HI * SH_LO)

_CACHE = {}


def _ensure_ntff_hook():
    """Install the axon NTFF profile hook shim (antenv.axon_hooks is absent
    in this image). Harmless if profiling is never requested."""
    if "antenv.axon_hooks" in sys.modules:
        return
    try:
        import antenv
        mod = types.ModuleType("antenv.axon_hooks")
        mod._hook = None
        mod.set_axon_ntff_profile_hook = lambda h: setattr(mod, "_hook", h)
        mod.get_axon_ntff_profile_hook = lambda: mod._hook
        sys.modules["antenv.axon_hooks"] = mod
        antenv.axon_hooks = mod
        from trn_agent_boot.trn_boot import _ntff_profile_via_ctypes
        mod._hook = _ntff_profile_via_ctypes("/opt/axon/libaxon_pjrt.so")
    except Exception:
        pass


def _build_nc():
    nc = bacc.Bacc(None)

    d = {}
    d["wqk8"] = nc.dram_tensor("wqk8", [4, 128, 8192], F8, kind="ExternalInput")
    d["wv8"] = nc.dram_tensor("wv8", [4, 128, 8192], F8, kind="ExternalInput")
    d["wo16"] = nc.dram_tensor("wo16", [4, 128, 8192], BF, kind="ExternalInput")
    d["hf8"] = nc.dram_tensor("hf8", [128, 64], F8, kind="ExternalInput")
    d["wab"] = nc.dram_tensor("wab", [128, 128], F32, kind="ExternalInput")
    d["hrep"] = nc.dram_tensor("hrep", [128, 128], F32, kind="ExternalInput")
    d["state16"] = nc.dram_tensor("state16", [128, 2048], BF, kind="ExternalInput")
    d["qkcache"] = nc.dram_tensor("qkcache", [128, 24], F32, kind="ExternalInput")
    d["qkconvw"] = nc.dram_tensor("qkconvw", [128, 32], F32, kind="ExternalInput")
    d["vcache"] = nc.dram_tensor("vcache", [128, 24], F32, kind="ExternalInput")
    d["vconvw"] = nc.dram_tensor("vconvw", [128, 32], F32, kind="ExternalInput")
    d["fsc32"] = nc.dram_tensor("fsc32", [2, 1], F32, kind="ExternalInput")
    d["eye16"] = nc.dram_tensor("eye16", [2, 2], BF, kind="ExternalInput")
    out_d = nc.dram_tensor("out", [2, H], F32, kind="ExternalOutput")

    with tile.TileContext(nc) as tc:
        with (
            tc.tile_pool(name="smalls", bufs=1) as sm,
            tc.tile_pool(name="wp8", bufs=4) as wp8,
            tc.tile_pool(name="wpo", bufs=4) as wpo,
            tc.tile_pool(name="psum", bufs=8, space="PSUM") as pm,
        ):
            def emit():
                # ---- small input DMAs (SWDGE keeps the HWDGE ring clear) ----
                hf8 = sm.tile([128, 2, 32], F8, tag="hf8")
                nc.gpsimd.dma_start(
                    out=hf8[:], in_=d["hf8"][:].rearrange("p (i m) -> p i m", i=2))
                wab = sm.tile([128, 128], F32, tag="wab")
                hrep = sm.tile([128, 128], F32, tag="hrep")
                st16 = sm.tile([128, 2048], BF, tag="st16")
                qkca = sm.tile([128, 24], F32, tag="qkca")
                qkcw = sm.tile([128, 32], F32, tag="qkcw")
                vca = sm.tile([128, 24], F32, tag="vca")
                vcw = sm.tile([128, 32], F32, tag="vcw")
                fsc = sm.tile([2, 1], F32, tag="fsc")
                eye = sm.tile([2, 2], BF, tag="eye")
                for t, src in [(wab, "wab"), (hrep, "hrep"), (fsc, "fsc32"),
                               (eye, "eye16"), (st16, "state16"),
                               (qkca, "qkcache"), (qkcw, "qkconvw"),
                               (vca, "vcache"), (vcw, "vconvw")]:
                    nc.gpsimd.dma_start(out=t[:], in_=d[src][:])
                ones = sm.tile([1, 128], F32, tag="ones")
                nc.vector.memset(ones[:], 1.0)
                onesc = sm.tile([128, 1], F32, tag="onesc")
                nc.vector.memset(onesc[:], 1.0)
                epst = sm.tile([1, 1], F32, tag="epst")
                nc.vector.memset(epst[:], EPS)

                # ---- psum tiles (allocation order fixes pool-slot reuse) ----
                ps_ab = pm.tile([1, 4], F32, tag="ps", name="ps_ab")
                ps_q = pm.tile([2, 512], F32, tag="ps", name="ps_q")
                ps_k = pm.tile([2, 512], F32, tag="ps", name="ps_k")
                ps_v0 = pm.tile([2, 512], F32, tag="ps", name="ps_v0")
                ps_v1 = pm.tile([2, 512], F32, tag="ps", name="ps_v1")
                t_col = pm.tile([128, 8], F32, tag="ps", name="t_col")
                t_row = pm.tile([1, 8], F32, tag="ps", name="t_row")
                ps_st0 = pm.tile([2, 512], F32, tag="ps", name="ps_st0")
                ps_st1 = pm.tile([2, 512], F32, tag="ps", name="ps_st1")
                ps_stc = pm.tile([128, 16], F32, tag="ps", name="ps_stc")

                # ---- sbuf chain tiles ----
                ab = sm.tile([1, 4], F32, tag="ab")
                qsb = sm.tile([2, 512], F32, tag="qsb")
                ksb = sm.tile([2, 512], F32, tag="ksb")
                qkcol = sm.tile([128, 8], F32, tag="qkcol")
                qacc = sm.tile([128, 8], F32, tag="qacc")
                qtmp = sm.tile([128, 8], F32, tag="qtmp")
                x1 = sm.tile([128, 8], F32, tag="x1")
                x116 = sm.tile([128, 8], BF, tag="x116")
                sq = sm.tile([128, 8], F32, tag="sq")
                ssr = sm.tile([1, 8], F32, tag="ssr")
                ssh = sm.tile([1, 4], F32, tag="ssh")
                rin = sm.tile([1, 4], F32, tag="rin")
                dm = sm.tile([128, 4], F32, tag="dm")
                dotr = sm.tile([1, 4], F32, tag="dotr")
                dot = sm.tile([1, 2], F32, tag="dot")
                aq2 = sm.tile([1, 2], F32, tag="aq2")
                ak2 = sm.tile([1, 2], F32, tag="ak2")
                bd = sm.tile([1, 2], F32, tag="bd")
                bdak = sm.tile([1, 2], F32, tag="bdak")
                abc6 = sm.tile([128, 6], F32, tag="abc6")
                vacc = sm.tile([128, 8], F32, tag="vacc")
                vtmp = sm.tile([128, 8], F32, tag="vtmp")
                stsb0 = sm.tile([2, 512], BF, tag="stsb0")
                stsb1 = sm.tile([2, 512], BF, tag="stsb1")
                vsb = sm.tile([2, 1024], F32, tag="vsb")
                vcol = sm.tile([128, 8], F32, tag="vcol")
                v1c = sm.tile([128, 8], F32, tag="v1c")
                u1c = sm.tile([128, 8], F32, tag="u1c")
                u2c = sm.tile([128, 8], F32, tag="u2c")
                errc = sm.tile([128, 4], F32, tag="errc")
                t1c = sm.tile([128, 4], F32, tag="t1c")
                ov16 = sm.tile([128, 16], BF, tag="ov16")
                dum = sm.tile([1, 1], F32, tag="dum")
                out_sb = sm.tile([2, H], F32, tag="out_sb")

                # ---- injected work: alpha/beta + conv cache taps ----
                def pre_ab():
                    # hrep[p, 4cc+j] = h[cc*128+p]; wab[p, 4cc+j] = Wab[j, ...]
                    abm = sm.tile([128, 128], F32, tag="abm")
                    nc.vector.tensor_mul(abm[:], wab[:], hrep[:])
                    abr = sm.tile([128, 4], F32, tag="abr")
                    nc.vector.reduce_sum(
                        abr[:],
                        abm[:].rearrange("p (cc f) -> p f cc", f=4),
                        axis=mybir.AxisListType.X)
                    nc.tensor.matmul(ps_ab[0:1, :], onesc[:, 0:1], abr[:],
                                     start=True, stop=True)
                    nc.scalar.activation(ab[:], ps_ab[:], AF.Sigmoid)

                def pre_taps():
                    # q/k conv cache taps -> qacc; v conv cache taps -> vacc
                    nc.vector.tensor_mul(qacc[:], qkca[:, 0:8], qkcw[:, 0:8])
                    for tpi in (1, 2):
                        nc.vector.tensor_mul(qtmp[:], qkca[:, 8 * tpi:8 * tpi + 8],
                                             qkcw[:, 8 * tpi:8 * tpi + 8])
                        nc.vector.tensor_add(qacc[:], qacc[:], qtmp[:])
                    nc.vector.tensor_mul(vacc[:], vca[:, 0:8], vcw[:, 0:8])
                    for tpi in (1, 2):
                        nc.vector.tensor_mul(vtmp[:], vca[:, 8 * tpi:8 * tpi + 8],
                                             vcw[:, 8 * tpi:8 * tpi + 8])
                        nc.vector.tensor_add(vacc[:], vacc[:], vtmp[:])

                # ---- fp8 DoubleRow streaming matvec ----
                def stream8(dram, ps0, ps1, inject=None):
                    """dram [4, 128, 8192] fp8, layout (d, p, (pr two rh r)).
                    rh=0 -> ps0[2,512], rh=1 -> ps1[2,512], M=2 (h hi, lo)."""
                    last = None
                    for dd in range(4):
                        t = wp8.tile([128, 8192], F8, tag="w8", name="w8t")
                        nc.sync.dma_start(out=t[:], in_=dram[dd])
                        tv = t[:].rearrange(
                            "p (pr two rh r) -> p pr two rh r",
                            pr=4, two=2, r=512)
                        last = tv
                        for pr in range(4):
                            pair = 4 * dd + pr
                            lh = hf8[:, 0:2, 2 * pair:2 * pair + 2]
                            nc.tensor.matmul(
                                ps0[0:2, :], lh, tv[:, pr, 0:2, 0, :],
                                start=(pair == 0), stop=(pair == 15),
                                perf_mode=PM.DoubleRow)
                            nc.tensor.matmul(
                                ps1[0:2, :], lh, tv[:, pr, 0:2, 1, :],
                                start=(pair == 0), stop=(pair == 15),
                                perf_mode=PM.DoubleRow)
                        if inject and dd in inject:
                            inject[dd]()
                    return last

                def chain_pe_0():
                    # scaled hi/lo fold + row->column (K=2, f32)
                    for c in range(4):
                        nc.tensor.matmul(t_col[:, c:c + 1],
                                         ksb[0:2, 128 * c:128 * c + 128],
                                         fsc[0:2, 0:1], start=True, stop=True)
                        nc.tensor.matmul(t_col[:, 4 + c:5 + c],
                                         qsb[0:2, 128 * c:128 * c + 128],
                                         fsc[0:2, 0:1], start=True, stop=True)
                    # conv tap3 + silu in columns (raw, un-normalized)
                    nc.vector.tensor_mul(qtmp[:], t_col[:], qkcw[:, 24:32])
                    nc.vector.tensor_add(qtmp[:], qacc[:], qtmp[:])
                    nc.scalar.activation(x1[:], qtmp[:], AF.Silu)
                    nc.vector.tensor_copy(x116[:], x1[:])
                    nc.vector.tensor_mul(sq[:], x1[:], x1[:])

                def chain_pe_1():
                    # per-column sum of squares -> per-head 1/||.||
                    nc.tensor.matmul(t_row[0:1, :], onesc[:, 0:1], sq[:],
                                     start=True, stop=True)
                    nc.vector.reduce_sum(
                        ssh[0:1, 0:4],
                        t_row[0:1, :].rearrange("a (g t) -> a g t", t=2),
                        axis=mybir.AxisListType.X)
                    srt = sm.tile([1, 4], F32, tag="srt")
                    nc.scalar.activation(srt[:], ssh[:], AF.Sqrt,
                                         bias=epst[0:1, 0:1])
                    nc.vector.reciprocal(rin[:], srt[:])
                    # raw q.k dot per head
                    nc.vector.tensor_mul(dm[:], x1[:, 4:8], x1[:, 0:4])

                def chain_pe_2():
                    nc.tensor.matmul(t_row[0:1, 0:4], onesc[:, 0:1], dm[:],
                                     start=True, stop=True)
                    # state matvecs, batched: lhsT = (k,q) column pairs of raw
                    # x1 (bf16), rhs = state rows -> ps_st[hh] rows (ks, qs)
                    xv = x116[:].rearrange("p (g c) -> p c g", c=4)
                    for hh in range(HPC):
                        pst = ps_st0 if hh == 0 else ps_st1
                        for d2 in range(2):
                            blk = 2 * hh + d2
                            nc.tensor.matmul(
                                pst[0:2, :], xv[:, 2 * hh + d2, 0:2],
                                st16[:, 512 * blk:512 * blk + 512],
                                start=(d2 == 0), stop=(d2 == 1))
                    nc.vector.reduce_sum(
                        dot[0:1, 0:2],
                        t_row[0:1, 0:4].rearrange("a (g t) -> a g t", t=2),
                        axis=mybir.AxisListType.X)
                    # per-head scalars: aq = a*rq, bd = b*dot_raw*rk*rq,
                    # bdak = bd*a*rk
                    nc.vector.tensor_mul(aq2[:], ab[0:1, 0:2], rin[0:1, 2:4])
                    nc.vector.tensor_mul(ak2[:], ab[0:1, 0:2], rin[0:1, 0:2])
                    nc.vector.tensor_mul(dot[:], dot[:], rin[0:1, 0:2])
                    nc.vector.tensor_mul(dot[:], dot[:], rin[0:1, 2:4])
                    nc.vector.tensor_mul(bd[:], ab[0:1, 2:4], dot[:])
                    nc.vector.tensor_mul(bdak[:], bd[:], ak2[:])
                    # silu table preload for the post-stream silu(v)
                    nc.scalar.activation(dum[:], epst[:], AF.Silu)
                    # broadcast [aq0 aq1 bdak0 bdak1 bd0 bd1] to 128 partitions
                    for j in range(2):
                        nc.tensor.matmul(t_col[:, j:j + 1], ones[0:1, :],
                                         aq2[0:1, j:j + 1], start=True, stop=True)
                        nc.tensor.matmul(t_col[:, 2 + j:3 + j], ones[0:1, :],
                                         bdak[0:1, j:j + 1], start=True, stop=True)
                        nc.tensor.matmul(t_col[:, 4 + j:5 + j], ones[0:1, :],
                                         bd[0:1, j:j + 1], start=True, stop=True)
                    nc.vector.tensor_copy(abc6[:], t_col[:, 0:6])
                    # fold state rows to columns and build the early half of
                    # the o-projection lhsT: u1 = aq*qs - bd*ak*ks
                    nc.vector.tensor_copy(stsb0[:], ps_st0[0:2, :])
                    nc.vector.tensor_copy(stsb1[:], ps_st1[0:2, :])
                    for hh in range(HPC):
                        ssb = stsb0 if hh == 0 else stsb1
                        for c in range(4):
                            nc.tensor.matmul(
                                ps_stc[:, 8 * hh + 2 * c:8 * hh + 2 * c + 2],
                                ssb[0:2, 128 * c:128 * c + 128],
                                eye[0:2, 0:2], start=True, stop=True)
                    stc = ps_stc[:].rearrange("p (hh c n) -> p hh n c", hh=2, n=2)
                    for hh in range(HPC):
                        nc.vector.tensor_scalar(out=t1c[:], in0=stc[:, hh, 1, :],
                                                scalar1=abc6[:, hh:hh + 1],
                                                scalar2=None, op0=OP.mult)
                        nc.vector.tensor_scalar(out=errc[:], in0=stc[:, hh, 0, :],
                                                scalar1=abc6[:, 2 + hh:3 + hh],
                                                scalar2=None, op0=OP.mult)
                        nc.vector.tensor_sub(u1c[:, 4 * hh:4 * hh + 4],
                                             t1c[:], errc[:])
                    # u1 -> even lhsT columns of the o-projection
                    nc.vector.tensor_copy(
                        ov16[:].rearrange("p (j m) -> p m j", m=2)[:, 0, :],
                        u1c[:])

                # ---- phase 1: q/k matvec (rh=0 -> q rows, rh=1 -> k rows) ----
                stream8(d["wqk8"], ps_q, ps_k,
                        inject={0: pre_ab, 1: pre_taps})
                nc.vector.tensor_copy(qsb[:], ps_q[0:2, :])
                nc.vector.tensor_copy(ksb[:], ps_k[0:2, :])

                # ---- phase 2: v matvec with injected chain ----
                vlast = stream8(d["wv8"], ps_v0, ps_v1,
                                inject={0: chain_pe_0, 1: chain_pe_1,
                                        2: chain_pe_2})

                # ---- phase 3: post-stream chain (v only), pipelined per head
                # so the o-projection can start on head0's tiles sooner ----
                nc.vector.tensor_copy(vsb[0:2, 0:512], ps_v0[0:2, :])
                nc.scalar.copy(vsb[0:2, 512:1024], ps_v1[0:2, :])
                ovv = ov16[:].rearrange("p (j m) -> p m j", m=2)
                for hh in range(HPC):
                    for c in range(4):
                        j = 4 * hh + c
                        nc.tensor.matmul(t_col[:, j:j + 1],
                                         vsb[0:2, 128 * j:128 * j + 128],
                                         fsc[0:2, 0:1], start=True, stop=True)
                    sl = slice(4 * hh, 4 * hh + 4)
                    nc.vector.tensor_mul(vtmp[:, sl], t_col[:, sl],
                                         vcw[:, 24 + 4 * hh:28 + 4 * hh])
                    nc.vector.tensor_add(vtmp[:, sl], vacc[:, sl], vtmp[:, sl])
                    nc.scalar.activation(v1c[:, sl], vtmp[:, sl], AF.Silu)
                    nc.vector.tensor_scalar(out=u2c[:, sl], in0=v1c[:, sl],
                                            scalar1=abc6[:, 4 + hh:5 + hh],
                                            scalar2=None, op0=OP.mult)
                    nc.vector.tensor_copy(ovv[:, 1, sl], u2c[:, sl])

                # ---- phase 4: output projection (bf16, M=2: u1, u2) ----
                ps_o = [pm.tile([2, 512], F32, tag="ps", name=f"ps_o{i}")
                        for i in range(8)]
                for dd in range(4):
                    t = wpo.tile([128, 8192], BF, tag="wo", name="wot")
                    nc.sync.dma_start(out=t[:], in_=d["wo16"][dd])
                    for i in range(2):
                        j = 2 * dd + i
                        for it in range(8):
                            nc.tensor.matmul(
                                ps_o[it][0:2, :], ov16[:, 2 * j:2 * j + 2],
                                t[:, 4096 * i + 512 * it:4096 * i + 512 * it + 512],
                                start=(j == 0), stop=(j == 7))
                for it in range(8):
                    dst = out_sb[0:2, 512 * it:512 * it + 512]
                    if it % 2 == 0:
                        nc.vector.tensor_copy(dst, ps_o[it][0:2, :])
                    else:
                        nc.scalar.copy(dst, ps_o[it][0:2, :])
                nc.sync.dma_start(out=out_d[:], in_=out_sb[:])

            emit()

    nc.finalize()
    return nc


def _prep_in_maps(inputs):
    f32 = np.float32
    hid = np.asarray(inputs["hidden_states"], f32)[0, :, 0, 0]     # [4096]
    Wq = np.asarray(inputs["Wq"], f32)
    Wk = np.asarray(inputs["Wk"], f32)
    Wv = np.asarray(inputs["Wv"], f32)
    Wo = np.asarray(inputs["Wo"], f32)
    Wa = np.asarray(inputs["Wa"], f32)
    Wb = np.asarray(inputs["Wb"], f32)
    qcw = np.asarray(inputs["q_conv_w"], f32)[0]                   # [QK, 4]
    kcw = np.asarray(inputs["k_conv_w"], f32)[0]
    vcw = np.asarray(inputs["v_conv_w"], f32)[0]                   # [VD, 4]
    qca = np.asarray(inputs["q_cache"], f32)[0]                    # [QK, 3]
    kca = np.asarray(inputs["k_cache"], f32)[0]
    vca = np.asarray(inputs["v_cache"], f32)[0]                    # [VD, 3]
    state = np.asarray(inputs["state"], f32)[0]                    # [16,256,512]

    # h fp8 hi/lo pair, [128, 64]: col (two*32 + pair*2 + m),
    # value row j = (pair*2 + two)*128 + p
    h_hi8 = (hid * SH_HI).astype(E4)
    h_lo8 = ((hid * SH_HI - h_hi8.astype(f32)) * SH_LO).astype(E4)
    hp = np.stack([h_hi8, h_lo8], -1).reshape(16, 2, 128, 2)  # pair,two,p,m
    hf8 = np.ascontiguousarray(
        hp.transpose(2, 1, 0, 3).reshape(128, 64))            # p,(two pair m)

    # h replicated x4 for the DVE alpha/beta matvec: hrep[p, 4cc+j] = h[cc*128+p]
    hrep = np.ascontiguousarray(
        np.repeat(hid.reshape(32, 128).T[:, :, None], 4, axis=2).reshape(128, 128))

    def pack8(wt):
        """wt [4096, 1024] fp8 (contraction-major) -> [4, 128, 8192] with
        tile layout (d, p, (pr two rh r))."""
        a = wt.reshape(4, 4, 2, 128, 2, 512)      # d pr two p rh r
        return np.ascontiguousarray(
            a.transpose(0, 3, 1, 2, 4, 5).reshape(4, 128, 8192))

    in_maps = []
    for c in range(NCORES):
        rq = slice(c * RQ, (c + 1) * RQ)
        rv = slice(c * RV, (c + 1) * RV)
        wqk = np.concatenate([Wq[rq], Wk[rq]], axis=0)             # [1024, 4096]
        wqk8 = pack8(np.ascontiguousarray((wqk.T * SW)).astype(E4))
        wv8 = pack8(np.ascontiguousarray((Wv[rv].T * SW)).astype(E4))
        # Wo columns rv, transposed [1024, 4096] bf16, tiles (d, p, (i r))
        wot = np.ascontiguousarray(Wo[:, rv].T).astype(BF16)
        wo16 = np.ascontiguousarray(
            wot.reshape(4, 2, 128, 4096).transpose(0, 2, 1, 3).reshape(4, 128, 8192))

        wab = np.concatenate([Wa[2 * c:2 * c + 2], Wb[2 * c:2 * c + 2]], 0)
        wab_sb = np.ascontiguousarray(
            wab.reshape(4, 32, 128).transpose(2, 1, 0).reshape(128, 128))
        st_sb = np.ascontiguousarray(
            state[2 * c:2 * c + 2].reshape(2, 2, 128, 512)
            .transpose(2, 0, 1, 3).reshape(128, 2048)).astype(BF16)

        # q/k conv in column layout [128, 8*taps]: per tap, cols 0-3 = k
        # chunks (k idx 128c+p), cols 4-7 = q chunks
        qk_ca = np.concatenate(
            [np.concatenate([kca[rq, t].reshape(4, 128).T,
                             qca[rq, t].reshape(4, 128).T], 1)
             for t in range(3)], 1)
        qk_cw = np.concatenate(
            [np.concatenate([kcw[rq, t].reshape(4, 128).T,
                             qcw[rq, t].reshape(4, 128).T], 1)
             for t in range(4)], 1)
        # v conv in column layout [128, 8*taps]: vcol[p, 8t+cc] = v[128cc+p, t]
        v_ca = np.ascontiguousarray(
            vca[rv].reshape(8, 128, 3).transpose(1, 2, 0).reshape(128, 24))
        v_cw = np.ascontiguousarray(
            vcw[rv].reshape(8, 128, 4).transpose(1, 2, 0).reshape(128, 32))

        in_maps.append({
            "wqk8": wqk8, "wv8": wv8, "wo16": wo16,
            "hf8": hf8, "wab": wab_sb, "hrep": hrep, "state16": st_sb,
            "qkcache": np.ascontiguousarray(qk_ca),
            "qkconvw": np.ascontiguousarray(qk_cw),
            "vcache": v_ca, "vconvw": v_cw,
            "fsc32": np.array([[FS_HI], [FS_LO]], f32),
            "eye16": np.eye(2, dtype=f32).astype(BF16),
        })
    return in_maps


def _run(inputs, trace=False, tmpdir=None):
    _ensure_ntff_hook()
    if "nc" not in _CACHE:
        _CACHE["nc"] = _build_nc()
    nc = _CACHE["nc"]
    in_maps = _prep_in_maps(inputs)
    res = run_bass_kernel_spmd(nc, in_maps, list(range(NCORES)),
                               trace=trace, tmpdir=tmpdir)
    acc = np.zeros(H, np.float64)
    for c in range(NCORES):
        r = res.results[c]["out"].astype(np.float64)
        acc += r[0] + r[1]
    out = acc.astype(np.float32).reshape(1, H, 1, 1)
    return out, res


def kernel(**inputs):
    out, _ = _run(inputs, trace=False)
    return out


def kernel_traced(tmpdir=None, **inputs):
    return _run(inputs, trace=True, tmpdir=tmpdir)
